# revision 1
# baseline (speedup 1.0000x reference)
"""Trainium2 Bass kernel for single-head decoder attention.

Problem: B=8, S=2048, E=1024, D=128, O=1024 (fp32)
    q = query @ Wq + bq ; k = key @ Wk + bk ; v = value @ Wv + bv
    scores = (q @ k.T) / sqrt(D), causal-masked, softmax over keys
    out = (softmax @ v) @ Wo + bo

Sharding: data-parallel over batch, one batch element per NeuronCore (8 cores).

Per-core dataflow (fully fused, pipelined by 512-wide q column groups):
  - Host pre-transposes query/key/value to [E, S] (E-contraction needs E on
    SBUF partitions for both operands) and casts activations/weights to fp16:
    fp16 matmuls run at full PE rate on this toolchain (fp32/fp32r matmuls pay
    a ~10-20us/instruction weight-load penalty) and halve input DMA bytes.
    fp16's 11-bit mantissa keeps the end-to-end error ~3e-4, on par with
    fp32r; all intermediate values here are O(1e3) max, well within range.
  - group n: project q/k/v columns [n*512,(n+1)*512) (PSUM fp32, bias fused
    into the ACT eviction), PE-transpose v block, then attention superblock
    s=n, then the (s-1) output projection. Input DMA of group n+1 overlaps.
  - scores are computed TRANSPOSED: S_T[k, q] = kT_block.T @ qT_chunk, so the
    exp'd P_T[k, q] is directly the stationary operand for the PV matmul
    (no per-block P transposes).
  - softmax: no max-subtraction (scores are O(5) here, exp is safe in fp32
    PSUM); row sums via a ones-vector matmul over P_T; 1/rowsum commutes
    through the output projection and is applied as a per-partition scale on
    the final eviction.
  - causal mask: additive -1e30 tiles on diagonal 128x512 chunks only;
    strictly-upper blocks are never computed (~44% of attention skipped).
  - bv and bo fold into one host-side bias added after gather (softmax rows
    sum to 1):  attn @ (V + 1 bv^T) @ Wo + bo = attn @ V @ Wo + (bv@Wo + bo).
"""

import numpy as np

import concourse.bacc as bacc
import concourse.mybir as mybir
import concourse.tile as tile
from concourse.bass_utils import run_bass_kernel_spmd
from concourse.masks import make_identity

B, S, E, D, O = 8, 2048, 1024, 128, 1024
NCORES = 8
ET = E // 128          # 8 e-tiles
NSB = S // 512         # 4 q superblocks of 512
NQT = S // 128         # 16 q/k tiles of 128
SCALE = 1.0 / np.sqrt(D)
NEG = -1.0e30

F32 = mybir.dt.float32
DTYPE_MODE = "fp16"    # "fp16" | "bf16" | "f32r" | "fp32"

_prog_cache: dict = {}


def _mdt(dtype_mode):
    return {
        "fp16": mybir.dt.float16,
        "bf16": mybir.dt.bfloat16,
        "f32r": mybir.dt.float32r,
        "fp32": mybir.dt.float32,
    }[dtype_mode]


def _np_mdt(dtype_mode):
    import ml_dtypes
    return {
        "fp16": np.float16,
        "bf16": ml_dtypes.bfloat16,
        "f32r": np.float32,
        "fp32": np.float32,
    }[dtype_mode]


def _build(mode: str, dtype_mode: str, repeat: int = 1):
    """mode: 'causal' | 'full' | 'general'.

    repeat > 1 wraps the whole pipeline in a hardware For_i loop (same data
    each iteration) — used only for steady-state timing measurements.
    """
    MDT = _mdt(dtype_mode)
    two_byte = dtype_mode in ("fp16", "bf16")
    nc = bacc.Bacc("TRN2", target_bir_lowering=False, debug=False)

    xq = nc.dram_tensor("xq", [E, S], MDT, kind="ExternalInput").ap()
    xk = nc.dram_tensor("xk", [E, S], MDT, kind="ExternalInput").ap()
    xv = nc.dram_tensor("xv", [E, S], MDT, kind="ExternalInput").ap()
    wq = nc.dram_tensor("wq", [E, D], MDT, kind="ExternalInput").ap()
    wk = nc.dram_tensor("wk", [E, D], MDT, kind="ExternalInput").ap()
    wv = nc.dram_tensor("wv", [E, D], MDT, kind="ExternalInput").ap()
    wo = nc.dram_tensor("wo", [D, O], MDT, kind="ExternalInput").ap()
    bq = nc.dram_tensor("bq", [D, 1], F32, kind="ExternalInput").ap()
    bk = nc.dram_tensor("bk", [D, 1], F32, kind="ExternalInput").ap()
    ones = nc.dram_tensor("ones", [128, 1], MDT, kind="ExternalInput").ap()
    if mode == "causal":
        trineg = nc.dram_tensor("trineg", [4, 128, 512], F32, kind="ExternalInput").ap()
    if mode == "general":
        biasT = nc.dram_tensor("biasT", [S, S], F32, kind="ExternalInput").ap()
    # fp16 output when the compute dtype is 2-byte: halves out-DMA bytes; the
    # host upcasts and applies the (exact, fp32) folded bias afterwards.
    ODT = MDT if two_byte else F32
    out = nc.dram_tensor("out", [S, O], ODT, kind="ExternalOutput").ap()

    Ident = mybir.ActivationFunctionType.Identity
    Copy = mybir.ActivationFunctionType.Copy
    Exp = mybir.ActivationFunctionType.Exp

    def kmax_of(s):
        return 4 * s + 4 if mode == "causal" else NQT

    with tile.TileContext(nc) as tc:
        with (
            tc.tile_pool(name="const", bufs=1) as const,
            tc.tile_pool(name="pers", bufs=1) as pers,
            tc.tile_pool(name="ptp", bufs=28) as ptp,
            tc.tile_pool(name="xstage", bufs=32) as xstage,
            tc.tile_pool(name="vstage", bufs=2) as vstage,
            tc.tile_pool(name="outst", bufs=6) as outst,
            tc.tile_pool(name="bstage", bufs=4) as bstage,
            tc.tile_pool(name="ps_big", bufs=4, space="PSUM") as ps_big,
            tc.tile_pool(name="ps_ot", bufs=2, space="PSUM") as ps_ot,
            tc.tile_pool(name="ps_rs", bufs=1, space="PSUM") as ps_rs,
            tc.tile_pool(name="ps_vt", bufs=1, space="PSUM") as ps_vt,
        ):
            # ---- constants ----
            wq_sb = const.tile([128, ET, D], MDT)
            wk_sb = const.tile([128, ET, D], MDT)
            wv_sb = const.tile([128, ET, D], MDT)
            for w_sb, w_ap in ((wq_sb, wq), (wk_sb, wk), (wv_sb, wv)):
                nc.sync.dma_start(out=w_sb, in_=w_ap.rearrange("(e p) d -> p e d", p=128))
            wo_sb = const.tile([128, O], MDT)
            nc.sync.dma_start(out=wo_sb, in_=wo)
            bq_sb = const.tile([D, 1], F32)
            nc.sync.dma_start(out=bq_sb, in_=bq)
            bk_sb = const.tile([D, 1], F32)
            nc.sync.dma_start(out=bk_sb, in_=bk)
            ones_sb = const.tile([128, 1], MDT)
            nc.sync.dma_start(out=ones_sb, in_=ones)
            ones32_sb = const.tile([128, 1], F32)
            nc.vector.memset(ones32_sb, 1.0)
            # identity + transpose path dtype: MDT when 2-byte (fast), else F32
            TDT = MDT if two_byte else F32
            ident = const.tile([128, 128], TDT)
            make_identity(nc, ident)
            if mode == "causal":
                tri_sb = const.tile([128, 4, 512], F32)
                nc.sync.dma_start(out=tri_sb, in_=trineg.rearrange("r p q -> p r q"))

            # ---- persistent tensors ----
            qT = pers.tile([D, S], MDT)       # [D, S]
            kT = pers.tile([D, S], MDT)
            v_all = pers.tile([128, NQT, D], MDT)  # [s-part, kj, D]
            oT = pers.tile([D, S], MDT)       # unnormalized (attn @ V).T
            rs_sb = pers.tile([1, S], MDT if two_byte else F32)  # row sums
            recip_sb = pers.tile([128, NQT], F32)

            # ---- pipeline ----
            def projection_cols(x_ap, w_sb, n):
                """Return PSUM chunk = (W.T @ x)[:, n*512:(n+1)*512]."""
                chunk = ps_big.tile([128, 512], F32, tag="big", name="pj")
                for e in range(ET):
                    xt = xstage.tile([128, 512], MDT, tag="xt", name="xt")
                    nc.sync.dma_start(
                        out=xt,
                        in_=x_ap[e * 128:(e + 1) * 128, n * 512:(n + 1) * 512])
                    nc.tensor.matmul(
                        chunk, w_sb[:, e, :], xt,
                        start=(e == 0), stop=(e == ET - 1))
                return chunk

            def emit_c(s):
                """Output projection for superblock s's 4 q-tiles."""
                for j in range(4):
                    i = 4 * s + j
                    p0 = ps_big.tile([128, 512], F32, tag="big", name="c0")
                    p1 = ps_big.tile([128, 512], F32, tag="big", name="c1")
                    lhs = oT[:, i * 128:(i + 1) * 128]
                    nc.tensor.matmul(p0, lhs, wo_sb[:, :512], start=True, stop=True)
                    nc.tensor.matmul(p1, lhs, wo_sb[:, 512:], start=True, stop=True)
                    ob = outst.tile([128, O], ODT, tag="ob", name="ob")
                    nc.scalar.mul(ob[:, :512], p0, recip_sb[:, i:i + 1])
                    nc.vector.tensor_scalar_mul(ob[:, 512:], p1, recip_sb[:, i:i + 1])
                    # out-DMA on SWDGE (gpsimd): keeps the SP sequencer free to
                    # trigger the next group's input DMAs without blocking.
                    nc.gpsimd.dma_start(out=out[i * 128:(i + 1) * 128, :], in_=ob)

            def _emit_proj_qk(n):
                csl = slice(n * 512, (n + 1) * 512)
                for x_ap, w_sb, dest, b_sb in (
                        (xq, wq_sb, qT, bq_sb), (xk, wk_sb, kT, bk_sb)):
                    chunk = projection_cols(x_ap, w_sb, n)
                    nc.scalar.activation(
                        out=dest[:, csl], in_=chunk, func=Ident, bias=b_sb, scale=1.0)

            def _emit_proj_v(n):
                vchunk = projection_cols(xv, wv_sb, n)
                vt_c = vstage.tile([128, 512], TDT, tag="vtc", name="vt_c")
                nc.scalar.activation(out=vt_c, in_=vchunk, func=Copy)
                for j in range(4):
                    kj = 4 * n + j
                    vt_ps = ps_vt.tile([128, 128], TDT, tag="vt", name="vt_ps")
                    nc.tensor.transpose(vt_ps, vt_c[:, j * 128:(j + 1) * 128], ident)
                    nc.vector.tensor_copy(v_all[:, kj, :], vt_ps)

            def _emit_scores(s):
                """S_T + exp chunks for superblock s; returns pts list."""
                kmax = kmax_of(s)
                qs = qT[:, s * 512:(s + 1) * 512]
                rs_ps = ps_rs.tile([1, 512], F32, tag="rs", name="rs_ps")
                pts = []
                for kj in range(kmax):
                    st = ps_big.tile([128, 512], F32, tag="big", name="st")
                    nc.tensor.matmul(
                        st, kT[:, kj * 128:(kj + 1) * 128], qs, start=True, stop=True)
                    if mode == "causal" and kj >= 4 * s:
                        # only columns < (r+1)*128 need masking: cols [0,r*128)
                        # are fully masked, [r*128,(r+1)*128) is the triangle,
                        # and everything after is fully valid.
                        r = kj - 4 * s
                        w = (r + 1) * 128
                        nc.vector.tensor_add(
                            st[:, :w], st[:, :w], tri_sb[:, r, :w])
                    elif mode == "general":
                        bt = bstage.tile([128, 512], F32, tag="bias", name="bt")
                        nc.sync.dma_start(
                            out=bt,
                            in_=biasT[kj * 128:(kj + 1) * 128, s * 512:(s + 1) * 512])
                        nc.vector.tensor_add(st, st, bt)
                    pt = ptp.tile([128, 512], MDT, tag="pt", name="pt")
                    nc.scalar.activation(out=pt, in_=st, func=Exp, scale=SCALE)
                    pts.append(pt)
                # row sums first: their eviction/transpose/reciprocal chain runs
                # on DVE while the O.T accumulation still streams on PE.
                for kj in range(kmax):
                    nc.tensor.matmul(
                        rs_ps, ones_sb, pts[kj],
                        start=(kj == 0), stop=(kj == kmax - 1))
                nc.vector.tensor_copy(rs_sb[:, s * 512:(s + 1) * 512], rs_ps)
                rsT_ps = ps_vt.tile([128, 4], F32, tag="vt", name="rsT_ps")
                for j in range(4):
                    i = 4 * s + j
                    nc.tensor.matmul(
                        rsT_ps[:, j:j + 1], rs_sb[:1, i * 128:(i + 1) * 128],
                        ones_sb[:1, :1] if two_byte else ones32_sb[:1, :1],
                        start=True, stop=True)
                nc.vector.tensor_scalar_add(recip_sb[:, 4 * s:4 * s + 4], rsT_ps, 1e-30)
                nc.vector.reciprocal(
                    recip_sb[:, 4 * s:4 * s + 4], recip_sb[:, 4 * s:4 * s + 4])
                return pts

            def _emit_ot(s, pts):
                kmax = kmax_of(s)
                ot_ps = ps_ot.tile([128, 512], F32, tag="ot", name="ot_ps")
                for kj in range(kmax):
                    nc.tensor.matmul(
                        ot_ps, v_all[:, kj, :], pts[kj],
                        start=(kj == 0), stop=(kj == kmax - 1))
                nc.vector.tensor_copy(oT[:, s * 512:(s + 1) * 512], ot_ps)

            def _emit_pipeline():
                if mode == "causal":
                    # superblock s only needs qT/kT cols < (s+1)*512 and V
                    # tiles <= 4s+3, so attention interleaves with projection
                    # groups. Scores (which need only q/k) are emitted BEFORE
                    # the v projection so the in-order PE streams score work
                    # while xv DMA is still in flight; C is deferred one
                    # superblock so out-DMAs queue behind the next group's
                    # input DMAs.
                    for s in range(NSB):
                        _emit_proj_qk(s)
                        pts = _emit_scores(s)
                        _emit_proj_v(s)
                        _emit_ot(s, pts)
                        if s > 0:
                            emit_c(s - 1)
                    emit_c(NSB - 1)
                else:
                    # non-causal: every superblock reads all of kT/V; project
                    # everything first.
                    for n in range(NSB):
                        _emit_proj_qk(n)
                        _emit_proj_v(n)
                    for s in range(NSB):
                        pts = _emit_scores(s)
                        _emit_ot(s, pts)
                        if s > 0:
                            emit_c(s - 1)
                    emit_c(NSB - 1)

            import contextlib
            loop_cm = (tc.For_i(0, repeat, 1) if repeat > 1
                       else contextlib.nullcontext())
            with loop_cm:
                _emit_pipeline()

    nc.compile()
    return nc


def _get_program(mode: str, dtype_mode: str, repeat: int = 1):
    key = (mode, dtype_mode, repeat)
    if key not in _prog_cache:
        _prog_cache[key] = _build(mode, dtype_mode, repeat)
    return _prog_cache[key]


def _tri_neg() -> np.ndarray:
    """trineg[r, k, q] = 0 if q >= r*128 + k else -1e30   (shape [4, 128, 512])"""
    r = np.arange(4)[:, None, None]
    k = np.arange(128)[None, :, None]
    q = np.arange(512)[None, None, :]
    return np.where(q >= r * 128 + k, 0.0, NEG).astype(np.float32)


def build_in_maps(inputs: dict, mode: str, dtype_mode: str):
    """Host-side layout prep shared by kernel() and the test harness."""
    ndt = _np_mdt(dtype_mode)
    query = np.asarray(inputs["query"], dtype=np.float32)
    key = np.asarray(inputs["key"], dtype=np.float32)
    value = np.asarray(inputs["value"], dtype=np.float32)
    xqT = np.ascontiguousarray(query.transpose(0, 2, 1)).astype(ndt)
    xkT = np.ascontiguousarray(key.transpose(0, 2, 1)).astype(ndt)
    xvT = np.ascontiguousarray(value.transpose(0, 2, 1)).astype(ndt)
    common = {
        "wq": np.asarray(inputs["Wq"], np.float32).astype(ndt),
        "wk": np.asarray(inputs["Wk"], np.float32).astype(ndt),
        "wv": np.asarray(inputs["Wv"], np.float32).astype(ndt),
        "wo": np.asarray(inputs["Wo"], np.float32).astype(ndt),
        "bq": np.asarray(inputs["bq"], np.float32).reshape(D, 1),
        "bk": np.asarray(inputs["bk"], np.float32).reshape(D, 1),
        "ones": np.ones((128, 1), np.float32).astype(ndt),
    }
    if mode == "causal":
        common["trineg"] = _tri_neg()
    if mode == "general":
        mask2 = (np.asarray(inputs["mask"]).reshape(-1, S, S)[0] != 0)
        common["biasT"] = np.ascontiguousarray(
            np.where(mask2, 0.0, NEG).astype(np.float32).T)
    return [{**common, "xq": xqT[b], "xk": xkT[b], "xv": xvT[b]}
            for b in range(B)]


def detect_mode(mask) -> str:
    mask2 = (np.asarray(mask).reshape(-1, S, S)[0] != 0)
    if np.array_equal(mask2, np.tril(np.ones((S, S), dtype=bool))):
        return "causal"
    if mask2.all():
        return "full"
    return "general"


def kernel(**inputs) -> np.ndarray:
    mode = detect_mode(inputs["mask"])
    nc = _get_program(mode, DTYPE_MODE)
    in_maps = build_in_maps(inputs, mode, DTYPE_MODE)

    bv = np.asarray(inputs["bv"], dtype=np.float32)
    bo = np.asarray(inputs["bo"], dtype=np.float32)
    Wo = np.asarray(inputs["Wo"], dtype=np.float32)
    bo_eff = (bv.astype(np.float64) @ Wo.astype(np.float64) + bo).astype(np.float32)

    try:
        res = run_bass_kernel_spmd(nc, in_maps, list(range(NCORES)))
    except Exception:
        # transient NRT/terminal failures have been observed to clear on retry
        import time as _time
        _time.sleep(20)
        res = run_bass_kernel_spmd(nc, in_maps, list(range(NCORES)))
    outs = np.stack(
        [np.asarray(res.results[b]["out"], dtype=np.float32) for b in range(B)],
        axis=0)
    outs += bo_eff[None, None, :]
    if mode == "general":
        # bv-folding assumes softmax rows sum to 1; fully-masked rows produce
        # all-zero attention (reference nan_to_num) and get only bo.
        mask2 = (np.asarray(inputs["mask"]).reshape(-1, S, S)[0] != 0)
        fully_masked = ~mask2.any(axis=1)
        if fully_masked.any():
            outs[:, fully_masked, :] = bo
    return outs.astype(np.float32)



# revision 4
# speedup vs baseline: 6.0406x; 6.0406x over previous
"""Trainium2 Bass kernel for single-head decoder attention.

Problem: B=8, S=2048, E=1024, D=128, O=1024 (fp32)
    q = query @ Wq + bq ; k = key @ Wk + bk ; v = value @ Wv + bv
    scores = (q @ k.T) / sqrt(D), causal-masked, softmax over keys
    out = (softmax @ v) @ Wo + bo
Sharding: data-parallel over batch, one batch element per NeuronCore (8 cores).

Per-core design (v2). The workload is simultaneously near three rooflines:
PE (~44us of fp16 matmul rows at 2.4GHz), DMA (~17MB at ~360GB/s/core =
~48us) and, in the v1 kernel, the SP sequencer (~104 dma_starts x 565ns =
~59us). v2 attacks all three:
  - Input DMA is batched: one [128, 8e, 512q] strided DMA per (tensor,
    column-group) = 12 big DMAs instead of 96 tile DMAs. All are issued
    up-front; the whole input (96KB/partition) stages in SBUF so the DMA
    engines free-run at full bandwidth while compute streams behind.
  - Softmax row-sums no longer run on PE (v1: a ones-vector matmul per
    score tile = 512 PE rows each, ~8.5us). DVE accumulates exp tiles into
    a [128,512] fp32 accumulator; PE does a single f32r ones-matmul per
    512-wide superblock for the final cross-partition reduction.
  - Causal diagonal tiles only compute valid columns: score/exp/PV/rowsum
    all restrict to q >= k (the fully-masked left part of diagonal k-tiles
    is skipped, not just masked). Mask constant shrinks to one [128,128]
    additive -1e30 triangle tile (v1 staged 1MB of fp32 mask rows).
  - scores are computed TRANSPOSED: S_T[k, q] = kT_block.T @ qT_chunk, so
    the exp'd P_T[k, q] is directly the stationary operand for the PV
    matmul (no per-block P transposes).
  - fp16 compute throughout (full PE rate, half DMA bytes; ~7e-4 rel err),
    1/rowsum applied as a per-partition scale at the output eviction,
    bv/bo folded into a host-side bias (softmax rows sum to 1).
"""

import numpy as np

import concourse.bacc as bacc
import concourse.mybir as mybir
import concourse.tile as tile
from concourse.bass_utils import run_bass_kernel_spmd
from concourse.masks import make_identity

B, S, E, D, O = 8, 2048, 1024, 128, 1024
NCORES = 8
ET = E // 128          # 8 e-tiles
NSB = S // 512         # 4 q superblocks of 512
NQT = S // 128         # 16 q/k tiles of 128
SCALE = 1.0 / np.sqrt(D)
NEG = -1.0e30

F32 = mybir.dt.float32
F32R = mybir.dt.float32r
DTYPE_MODE = "fp16"    # "fp16" | "bf16" | "f32r" | "fp32"

_prog_cache: dict = {}


def _mdt(dtype_mode):
    return {
        "fp16": mybir.dt.float16,
        "bf16": mybir.dt.bfloat16,
        "f32r": mybir.dt.float32r,
        "fp32": mybir.dt.float32,
    }[dtype_mode]


def _np_mdt(dtype_mode):
    import ml_dtypes
    return {
        "fp16": np.float16,
        "bf16": ml_dtypes.bfloat16,
        "f32r": np.float32,
        "fp32": np.float32,
    }[dtype_mode]


def _build(mode: str, dtype_mode: str, repeat: int = 1):
    """mode: 'causal' | 'full' | 'general'.

    repeat > 1 wraps the whole pipeline in a hardware For_i loop (same data
    each iteration) — used only for steady-state timing measurements.
    """
    MDT = _mdt(dtype_mode)
    two_byte = dtype_mode in ("fp16", "bf16")
    nc = bacc.Bacc("TRN2", target_bir_lowering=False, debug=False)

    xq = nc.dram_tensor("xq", [E, S], MDT, kind="ExternalInput").ap()
    xk = nc.dram_tensor("xk", [E, S], MDT, kind="ExternalInput").ap()
    xv = nc.dram_tensor("xv", [E, S], MDT, kind="ExternalInput").ap()
    wq = nc.dram_tensor("wq", [E, D], MDT, kind="ExternalInput").ap()
    wk = nc.dram_tensor("wk", [E, D], MDT, kind="ExternalInput").ap()
    wv = nc.dram_tensor("wv", [E, D], MDT, kind="ExternalInput").ap()
    wo = nc.dram_tensor("wo", [D, O], MDT, kind="ExternalInput").ap()
    bq = nc.dram_tensor("bq", [D, 1], F32, kind="ExternalInput").ap()
    bk = nc.dram_tensor("bk", [D, 1], F32, kind="ExternalInput").ap()
    if mode == "causal":
        # tri[k, c] = 0 if c >= k else -1e30: the one 128x128 diagonal block
        tri = nc.dram_tensor("tri", [128, 128], F32, kind="ExternalInput").ap()
    if mode == "general":
        biasT = nc.dram_tensor("biasT", [S, S], F32, kind="ExternalInput").ap()
    # fp16 output when the compute dtype is 2-byte: halves out-DMA bytes; the
    # host upcasts and applies the (exact, fp32) folded bias afterwards.
    ODT = MDT if two_byte else F32
    out = nc.dram_tensor("out", [S, O], ODT, kind="ExternalOutput").ap()

    Ident = mybir.ActivationFunctionType.Identity
    Copy = mybir.ActivationFunctionType.Copy
    Exp = mybir.ActivationFunctionType.Exp

    def kmax_of(s):
        return 4 * s + 4 if mode == "causal" else NQT

    def qoff_of(s, kj):
        """First valid q column (within the 512 superblock) for k-tile kj."""
        if mode == "causal" and kj >= 4 * s:
            return (kj - 4 * s) * 128
        return 0

    with tile.TileContext(nc) as tc:
        with (
            tc.tile_pool(name="const", bufs=1) as const,
            tc.tile_pool(name="pers", bufs=1) as pers,
            tc.tile_pool(name="ptp", bufs=24) as ptp,
            tc.tile_pool(name="xstage", bufs=3 * NSB) as xstage,
            tc.tile_pool(name="vstage", bufs=2) as vstage,
            tc.tile_pool(name="rsacc", bufs=2) as rsacc,
            tc.tile_pool(name="rsrow", bufs=2) as rsrow,
            tc.tile_pool(name="outst", bufs=4) as outst,
            tc.tile_pool(name="bstage", bufs=4) as bstage,
            tc.tile_pool(name="ps_big", bufs=4, space="PSUM") as ps_big,
            tc.tile_pool(name="ps_ot", bufs=2, space="PSUM") as ps_ot,
            tc.tile_pool(name="ps_vt", bufs=1, space="PSUM") as ps_vt,
            tc.tile_pool(name="ps_rs", bufs=1, space="PSUM") as ps_rs,
        ):
            # ---- constants ----
            wq_sb = const.tile([128, ET, D], MDT)
            wk_sb = const.tile([128, ET, D], MDT)
            wv_sb = const.tile([128, ET, D], MDT)
            for w_sb, w_ap in ((wq_sb, wq), (wk_sb, wk), (wv_sb, wv)):
                nc.sync.dma_start(out=w_sb, in_=w_ap.rearrange("(e p) d -> p e d", p=128))
            wo_sb = const.tile([128, O], MDT)
            nc.sync.dma_start(out=wo_sb, in_=wo)
            bq_sb = const.tile([D, 1], F32)
            nc.sync.dma_start(out=bq_sb, in_=bq)
            bk_sb = const.tile([D, 1], F32)
            nc.sync.dma_start(out=bk_sb, in_=bk)
            ones_sb = const.tile([128, 1], MDT)
            nc.vector.memset(ones_sb, 1.0)
            # identity + transpose path dtype: MDT when 2-byte (fast), else F32
            TDT = MDT if two_byte else F32
            ident = const.tile([128, 128], TDT)
            make_identity(nc, ident)
            if mode == "causal":
                tri_sb = const.tile([128, 128], F32)
                nc.sync.dma_start(out=tri_sb, in_=tri)

            # ---- persistent tensors ----
            qT = pers.tile([D, S], MDT)       # [D, S]
            kT = pers.tile([D, S], MDT)
            v_all = pers.tile([128, NQT, D], MDT)  # [s-part, kj, D]
            oT = pers.tile([D, S], MDT)       # unnormalized (attn @ V).T
            recip_sb = pers.tile([128, NQT], F32)

            # ---- pipeline ----
            def stage_inputs():
                """One big strided DMA per (tensor, 512-col group): the full
                input stages in SBUF and the DMA engines free-run."""
                xg = {}
                for n in range(NSB):
                    for nm, x_ap in (("q", xq), ("k", xk), ("v", xv)):
                        xt = xstage.tile([128, ET, 512], MDT, tag="xt",
                                         name=f"x{nm}{n}")
                        nc.sync.dma_start(
                            out=xt,
                            in_=x_ap[:, n * 512:(n + 1) * 512].rearrange(
                                "(e p) q -> p e q", p=128))
                        xg[(nm, n)] = xt
                return xg

            def projection_cols(xt_g):
                """Return PSUM chunk = (W.T @ x)[:, n*512:(n+1)*512]."""
                def _emit(w_sb):
                    chunk = ps_big.tile([128, 512], F32, tag="big", name="pj")
                    for e in range(ET):
                        nc.tensor.matmul(
                            chunk, w_sb[:, e, :], xt_g[:, e, :],
                            start=(e == 0), stop=(e == ET - 1))
                    return chunk
                return _emit

            def _emit_proj_qk(xg, n):
                csl = slice(n * 512, (n + 1) * 512)
                for nm, w_sb, dest, b_sb in (
                        ("q", wq_sb, qT, bq_sb), ("k", wk_sb, kT, bk_sb)):
                    chunk = projection_cols(xg[(nm, n)])(w_sb)
                    nc.scalar.activation(
                        out=dest[:, csl], in_=chunk, func=Ident, bias=b_sb, scale=1.0)

            def _emit_proj_v(xg, n):
                vchunk = projection_cols(xg[("v", n)])(wv_sb)
                vt_c = vstage.tile([128, 512], TDT, tag="vtc", name="vt_c")
                nc.scalar.activation(out=vt_c, in_=vchunk, func=Copy)
                for j in range(4):
                    kj = 4 * n + j
                    vt_ps = ps_vt.tile([128, 128], TDT, tag="vt", name="vt_ps")
                    nc.tensor.transpose(vt_ps, vt_c[:, j * 128:(j + 1) * 128], ident)
                    nc.vector.tensor_copy(v_all[:, kj, :], vt_ps)

            def _emit_scores(s):
                """S_T + exp chunks for superblock s; DVE accumulates row sums.

                Returns pts list. Diagonal k-tiles only compute columns
                q >= k (qoff..512); the left part is fully causal-masked."""
                kmax = kmax_of(s)
                qs = qT[:, s * 512:(s + 1) * 512]
                rs_acc = rsacc.tile([128, 512], MDT, tag="rs", name="rs_acc")
                pts = []
                for kj in range(kmax):
                    qo = qoff_of(s, kj)
                    st = ps_big.tile([128, 512], F32, tag="big", name="st")
                    nc.tensor.matmul(
                        st[:, qo:], kT[:, kj * 128:(kj + 1) * 128], qs[:, qo:],
                        start=True, stop=True)
                    if mode == "causal" and kj >= 4 * s:
                        # the 128 cols at qo hold the triangle; cols beyond
                        # qo+128 are fully valid.
                        nc.vector.tensor_add(
                            st[:, qo:qo + 128], st[:, qo:qo + 128], tri_sb)
                    elif mode == "general":
                        bt = bstage.tile([128, 512], F32, tag="bias", name="bt")
                        nc.sync.dma_start(
                            out=bt,
                            in_=biasT[kj * 128:(kj + 1) * 128, s * 512:(s + 1) * 512])
                        nc.vector.tensor_add(st, st, bt)
                    pt = ptp.tile([128, 512], MDT, tag="pt", name="pt")
                    nc.scalar.activation(
                        out=pt[:, qo:], in_=st[:, qo:], func=Exp, scale=SCALE)
                    # row-sum accumulation on DVE (v1 burned a 512-row PE
                    # ones-matmul per tile on this)
                    if kj == 0:
                        nc.vector.tensor_copy(rs_acc, pt)
                    else:
                        nc.vector.tensor_add(
                            rs_acc[:, qo:], rs_acc[:, qo:], pt[:, qo:])
                    pts.append(pt)
                # final cross-partition reduction + reciprocal, transposed to
                # per-partition layout via tiny matmuls.
                rs_ps = ps_rs.tile([1, 512], F32, tag="rsp", name="rs_ps")
                nc.tensor.matmul(
                    rs_ps, ones_sb, rs_acc,
                    start=True, stop=True)
                rs_row = rsrow.tile([1, 512], MDT, tag="rsr", name="rs_row")
                nc.vector.tensor_copy(rs_row, rs_ps)
                rsT_ps = ps_vt.tile([128, 4], F32, tag="vt", name="rsT_ps")
                for j in range(4):
                    i = 4 * s + j
                    nc.tensor.matmul(
                        rsT_ps[:, j:j + 1],
                        rs_row[:1, j * 128:(j + 1) * 128],
                        ones_sb[:1, :1],
                        start=True, stop=True)
                nc.vector.tensor_scalar_add(recip_sb[:, 4 * s:4 * s + 4], rsT_ps, 1e-30)
                nc.vector.reciprocal(
                    recip_sb[:, 4 * s:4 * s + 4], recip_sb[:, 4 * s:4 * s + 4])
                return pts

            def _emit_ot(s, pts):
                kmax = kmax_of(s)
                ot_ps = ps_ot.tile([128, 512], F32, tag="ot", name="ot_ps")
                for kj in range(kmax):
                    qo = qoff_of(s, kj)
                    nc.tensor.matmul(
                        ot_ps[:, qo:], v_all[:, kj, :], pts[kj][:, qo:],
                        start=(kj == 0), stop=(kj == kmax - 1),
                        skip_group_check=(qo > 0))
                nc.scalar.activation(out=oT[:, s * 512:(s + 1) * 512],
                                     in_=ot_ps, func=Copy)

            def emit_c(s):
                """Output projection for superblock s's 4 q-tiles."""
                for j in range(4):
                    i = 4 * s + j
                    p0 = ps_big.tile([128, 512], F32, tag="big", name="c0")
                    p1 = ps_big.tile([128, 512], F32, tag="big", name="c1")
                    lhs = oT[:, i * 128:(i + 1) * 128]
                    nc.tensor.matmul(p0, lhs, wo_sb[:, :512], start=True, stop=True)
                    nc.tensor.matmul(p1, lhs, wo_sb[:, 512:], start=True, stop=True)
                    ob = outst.tile([128, O], ODT, tag="ob", name="ob")
                    nc.scalar.mul(ob[:, :512], p0, recip_sb[:, i:i + 1])
                    nc.scalar.mul(ob[:, 512:], p1, recip_sb[:, i:i + 1])
                    # out-DMA on SWDGE (gpsimd): keeps the SP sequencer free
                    nc.gpsimd.dma_start(out=out[i * 128:(i + 1) * 128, :], in_=ob)

            def _emit_pipeline():
                xg = stage_inputs()
                if mode == "causal":
                    # superblock s only needs qT/kT cols < (s+1)*512 and V
                    # tiles <= 4s+3, so attention interleaves with projection
                    # groups. C is deferred one superblock so the recip chain
                    # and oT eviction have a full superblock of slack.
                    for s in range(NSB):
                        _emit_proj_qk(xg, s)
                        _emit_proj_v(xg, s)
                        pts = _emit_scores(s)
                        _emit_ot(s, pts)
                        if s > 0:
                            emit_c(s - 1)
                    emit_c(NSB - 1)
                else:
                    # non-causal: every superblock reads all of kT/V; project
                    # everything first.
                    for n in range(NSB):
                        _emit_proj_qk(xg, n)
                        _emit_proj_v(xg, n)
                    for s in range(NSB):
                        pts = _emit_scores(s)
                        _emit_ot(s, pts)
                        if s > 0:
                            emit_c(s - 1)
                    emit_c(NSB - 1)

            import contextlib
            loop_cm = (tc.For_i(0, repeat, 1) if repeat > 1
                       else contextlib.nullcontext())
            with loop_cm:
                _emit_pipeline()

    nc.compile()
    return nc


def _get_program(mode: str, dtype_mode: str, repeat: int = 1):
    key = (mode, dtype_mode, repeat)
    if key not in _prog_cache:
        _prog_cache[key] = _build(mode, dtype_mode, repeat)
    return _prog_cache[key]


def _tri_neg() -> np.ndarray:
    """tri[k, c] = 0 if c >= k else -1e30   (shape [128, 128])"""
    k = np.arange(128)[:, None]
    c = np.arange(128)[None, :]
    return np.where(c >= k, 0.0, NEG).astype(np.float32)


def build_in_maps(inputs: dict, mode: str, dtype_mode: str):
    """Host-side layout prep shared by kernel() and the test harness."""
    ndt = _np_mdt(dtype_mode)
    query = np.asarray(inputs["query"], dtype=np.float32)
    key = np.asarray(inputs["key"], dtype=np.float32)
    value = np.asarray(inputs["value"], dtype=np.float32)
    xqT = np.ascontiguousarray(query.transpose(0, 2, 1)).astype(ndt)
    xkT = np.ascontiguousarray(key.transpose(0, 2, 1)).astype(ndt)
    xvT = np.ascontiguousarray(value.transpose(0, 2, 1)).astype(ndt)
    common = {
        "wq": np.asarray(inputs["Wq"], np.float32).astype(ndt),
        "wk": np.asarray(inputs["Wk"], np.float32).astype(ndt),
        "wv": np.asarray(inputs["Wv"], np.float32).astype(ndt),
        "wo": np.asarray(inputs["Wo"], np.float32).astype(ndt),
        "bq": np.asarray(inputs["bq"], np.float32).reshape(D, 1),
        "bk": np.asarray(inputs["bk"], np.float32).reshape(D, 1),
    }
    if mode == "causal":
        common["tri"] = _tri_neg()
    if mode == "general":
        mask2 = (np.asarray(inputs["mask"]).reshape(-1, S, S)[0] != 0)
        common["biasT"] = np.ascontiguousarray(
            np.where(mask2, 0.0, NEG).astype(np.float32).T)
    return [{**common, "xq": xqT[b], "xk": xkT[b], "xv": xvT[b]}
            for b in range(B)]


def detect_mode(mask) -> str:
    mask2 = (np.asarray(mask).reshape(-1, S, S)[0] != 0)
    if np.array_equal(mask2, np.tril(np.ones((S, S), dtype=bool))):
        return "causal"
    if mask2.all():
        return "full"
    return "general"


def kernel(**inputs) -> np.ndarray:
    mode = detect_mode(inputs["mask"])
    nc = _get_program(mode, DTYPE_MODE)
    in_maps = build_in_maps(inputs, mode, DTYPE_MODE)

    bv = np.asarray(inputs["bv"], dtype=np.float32)
    bo = np.asarray(inputs["bo"], dtype=np.float32)
    Wo = np.asarray(inputs["Wo"], dtype=np.float32)
    bo_eff = (bv.astype(np.float64) @ Wo.astype(np.float64) + bo).astype(np.float32)

    try:
        res = run_bass_kernel_spmd(nc, in_maps, list(range(NCORES)))
    except Exception:
        # transient NRT/terminal failures have been observed to clear on retry
        import time as _time
        _time.sleep(20)
        res = run_bass_kernel_spmd(nc, in_maps, list(range(NCORES)))
    outs = np.stack(
        [np.asarray(res.results[b]["out"], dtype=np.float32) for b in range(B)],
        axis=0)
    outs += bo_eff[None, None, :]
    if mode == "general":
        # bv-folding assumes softmax rows sum to 1; fully-masked rows produce
        # all-zero attention (reference nan_to_num) and get only bo.
        mask2 = (np.asarray(inputs["mask"]).reshape(-1, S, S)[0] != 0)
        fully_masked = ~mask2.any(axis=1)
        if fully_masked.any():
            outs[:, fully_masked, :] = bo
    return outs.astype(np.float32)
